# revision 7
# baseline (speedup 1.0000x reference)
"""AltAttention distributed Bass kernel for 8 TRN2 NeuronCores.

Reference computation (B=2, N=2048, C=1024, H=16, HD=64):
    qkv = x @ qkv_w.T -> split q,k,v heads
    attn = softmax(q k^T * HD**-0.5 + alibi + key_padding_mask(-inf))
    out  = (attn @ v merged heads) @ proj_w.T + proj_b

Sharding: core i handles batch b = i//4 and the 4 heads hg*4..hg*4+3
(hg = i%4).  Each core computes a partial output projection (rows of
proj_w.T restricted to its heads' features); the host sums the 4
partials per batch.

On-device layout (per core):
    xT      [1024, 2048]  x[b].T
    wqkvT   [1024, 768]   qkv_w rows for this core's heads, transposed;
                          col order: Q(h0..h3) K(h0..h3) V(h0..h3), 64 each;
                          the attention scale is folded into the Q columns
    ealibiT [4*2048,2048] exp(alibi[b,h].T + (-1e30 where padding_mask[b,k]))
    pwT     [256, 1024]   proj_w[:, head cols].T
    out     [1024, 2048]  partial (out @ proj_w.T).T  (features x seq)

Scores are computed transposed (S^T[k,q]) so the softmax denominator
falls out of the AV matmul via a ones-column appended to V.  The alibi
add is replaced by exp(S+a) = exp(S)*exp(a): ScalarE does exp(S)
straight out of PSUM and VectorE multiplies by the host-precomputed
exp(alibi), which in bf16 runs in the DVE 2x perf mode.
"""

import contextlib

import numpy as np
import ml_dtypes

import concourse.bass as bass
import concourse.tile as tile
from concourse import mybir
from concourse.bass_utils import run_bass_kernel_spmd

B, N, C, H = 2, 2048, 1024, 16
HD = C // H
SCALE = HD ** -0.5
H_CORE = 4            # heads per core
NCORES = 8
F32 = mybir.dt.float32
BF16 = mybir.dt.bfloat16

NEG_MASK = -1e30

QB = 512              # q block (psum free dim per matmul)
KC = 128              # k chunk (psum partitions)
N_QB = N // QB        # 4
N_KC = N // KC        # 16

COMPUTE_DT = "bf16"   # "bf16" | "f32"


def _split_waits(nc, max_waits=1):
    """walrus in this container rejects instructions with >1 semaphore
    wait; hoist excess waits onto injected same-engine NOPs."""
    n_new = 0
    for f in nc.m.functions:
        for blk in f.blocks:
            new_insts = []
            for inst in blk.instructions:
                si = inst.sync_info
                if si is not None and si.on_wait and len(si.on_wait) > max_waits:
                    waits = list(si.on_wait)
                    extra, keep = waits[:-max_waits], waits[-max_waits:]
                    for j in range(0, len(extra), max_waits):
                        chunk = extra[j:j + max_waits]
                        nop = mybir.InstNoOp(
                            name=f"{inst.name}-waitsplit-{n_new}",
                            ins=[], outs=[],
                            sync_info=mybir.SyncInfo(on_wait=chunk, on_update=[]),
                        )
                        nop.engine = inst.engine
                        nc.register_instruction(nop)
                        new_insts.append(nop)
                        n_new += 1
                    si.on_wait = keep
                new_insts.append(inst)
            blk.instructions[:] = new_insts
    return n_new


def build_kernel(repeat=1, dt_name=COMPUTE_DT):
    DT = BF16 if dt_name == "bf16" else F32
    nc = bass.Bass()
    xT_e = nc.declare_dram_parameter("xT", [C, N], DT, isOutput=False)
    wqkvT_e = nc.declare_dram_parameter("wqkvT", [C, 3 * H_CORE * HD], DT, isOutput=False)
    ealibiT_e = nc.declare_dram_parameter("ealibiT", [H_CORE * N, N], DT, isOutput=False)
    pwT_e = nc.declare_dram_parameter("pwT", [H_CORE * HD, C], DT, isOutput=False)
    out_e = nc.declare_dram_parameter("out", [C, N], F32, isOutput=True)

    FQKV = 3 * H_CORE * HD    # 768
    Exp = mybir.ActivationFunctionType.Exp
    Ln = mybir.ActivationFunctionType.Ln

    with tile.TileContext(nc) as tc:
        rep_ctx = tc.For_i(0, repeat) if repeat > 1 else contextlib.nullcontext()
        # ---- persistent SBUF tensors ----
        with rep_ctx, tc.tile_pool(name="persist", bufs=1) as persist:
            qkT = persist.tile([128, 4 * N], DT)       # Q,K feature-major: 4 chunks
            v_sb = [persist.tile([128, N_KC, HD + 1], DT, name=f"v{h}") for h in range(H_CORE)]
            avt = [persist.tile([128, N], DT, name=f"avt{i}") for i in range(2)]
            pwT_sb = persist.tile([128, 2 * C], DT)
            ones_sb = persist.tile([128, HD], F32)     # epilogue bcast lhsT

            nc.vector.memset(ones_sb[:], 1.0)
            for h in range(H_CORE):
                nc.vector.memset(v_sb[h][:, :, HD], 1.0)
            for ic in range(2):
                nc.sync.dma_start(pwT_sb[:, ic * C:(ic + 1) * C],
                                  pwT_e[ic * 128:(ic + 1) * 128, :])

            # ---- phase 1: QKV projection ----
            with tc.tile_pool(name="xw", bufs=1) as xw, \
                 tc.tile_pool(name="psum_qkv", bufs=4, space="PSUM") as pq:
                xT_sb = xw.tile([128, 8 * N], DT)
                wq_sb = xw.tile([128, 8 * FQKV], DT)
                for cc in range(8):
                    nc.sync.dma_start(xT_sb[:, cc * N:(cc + 1) * N],
                                      xT_e[cc * 128:(cc + 1) * 128, :])
                    nc.sync.dma_start(wq_sb[:, cc * FQKV:(cc + 1) * FQKV],
                                      wqkvT_e[cc * 128:(cc + 1) * 128, :])

                # Q,K feature-major  [512 feats, N]
                for mc in range(4):
                    for nb in range(N_QB):
                        ps = pq.tile([128, QB], F32)
                        for cc in range(8):
                            nc.tensor.matmul(
                                ps[:],
                                lhsT=wq_sb[:, cc * FQKV + mc * 128: cc * FQKV + (mc + 1) * 128],
                                rhs=xT_sb[:, cc * N + nb * QB: cc * N + nb * QB + QB],
                                start=(cc == 0), stop=(cc == 7),
                            )
                        dst = qkT[:, mc * N + nb * QB: mc * N + nb * QB + QB]
                        if (mc + nb) % 2:
                            nc.scalar.copy(dst, ps[:])
                        else:
                            nc.vector.tensor_copy(dst, ps[:])

                # V sequence-major  [N, 256] -> per-head [N_KC, 128, HD+1]
                for kc in range(N_KC):
                    ps = pq.tile([128, H_CORE * HD], F32)
                    for cc in range(8):
                        nc.tensor.matmul(
                            ps[:],
                            lhsT=xT_sb[:, cc * N + kc * 128: cc * N + (kc + 1) * 128],
                            rhs=wq_sb[:, cc * FQKV + 512: (cc + 1) * FQKV],
                            start=(cc == 0), stop=(cc == 7),
                        )
                    for h in range(H_CORE):
                        if (kc + h) % 2 == 0:
                            nc.vector.tensor_copy(v_sb[h][:, kc, 0:HD],
                                                  ps[:, h * HD:(h + 1) * HD])
                        else:
                            nc.scalar.copy(v_sb[h][:, kc, 0:HD],
                                           ps[:, h * HD:(h + 1) * HD])

            # ---- phase 2: attention per head-pair x q-block ----
            # The two heads of a pair sit at partitions 0-63 / 64-127 of the
            # same qkT chunk, so their S^T matmuls run concurrently in the PE
            # via row tiling, writing the two banks of one [128, 2*QB] psum
            # tile.  exp and the ealibi multiply then process both heads in
            # one instruction each.
            with tc.tile_pool(name="alibi", bufs=6) as alp, \
                 tc.tile_pool(name="sexp", bufs=4) as sep, \
                 tc.tile_pool(name="pmul", bufs=4) as pmp, \
                 tc.tile_pool(name="stat", bufs=4) as stp, \
                 tc.tile_pool(name="avtmp", bufs=2) as avp, \
                 tc.tile_pool(name="psum_s", bufs=2, space="PSUM") as pss, \
                 tc.tile_pool(name="psum_av", bufs=2, space="PSUM") as pav, \
                 tc.tile_pool(name="psum_bc", bufs=2, space="PSUM") as pbc:
                for hp in range(2):           # head pair index
                    for qb in range(N_QB):
                        ps_av = [pav.tile([65, QB], F32, name=f"ps_av{p}", tag="ps_av")
                                 for p in range(2)]
                        for kc in range(N_KC):
                            ps_s = pss.tile([128, 2 * QB], F32)
                            al = alp.tile([128, 2 * QB], DT)
                            for par in range(2):     # head within pair
                                h = 2 * hp + par
                                p0, p1 = par * 64, par * 64 + 64
                                nc.tensor.matmul(
                                    ps_s[:, par * QB:(par + 1) * QB],
                                    lhsT=qkT[p0:p1, (2 + hp) * N + kc * KC: (2 + hp) * N + (kc + 1) * KC],
                                    rhs=qkT[p0:p1, hp * N + qb * QB: hp * N + qb * QB + QB],
                                    start=True, stop=True,
                                )
                                nc.sync.dma_start(
                                    al[:, par * QB:(par + 1) * QB],
                                    ealibiT_e[h * N + kc * KC: h * N + (kc + 1) * KC,
                                              qb * QB: qb * QB + QB])
                            sexp = sep.tile([128, 2 * QB], DT)
                            nc.scalar.activation(sexp[:], ps_s[:], Exp)
                            pm = pmp.tile([128, 2 * QB], DT)
                            nc.vector.tensor_tensor(pm[:], sexp[:], al[:],
                                                    mybir.AluOpType.mult)
                            for par in range(2):
                                h = 2 * hp + par
                                nc.tensor.matmul(
                                    ps_av[par][:],
                                    lhsT=v_sb[h][:, kc, :],
                                    rhs=pm[:, par * QB:(par + 1) * QB],
                                    start=(kc == 0), stop=(kc == N_KC - 1),
                                )
                        # epilogue: normalize by the ones-column denominator
                        for par in range(2):
                            st = stp.tile([65, 2 * QB], F32)
                            nc.scalar.activation(st[64:65, 0:QB], ps_av[par][64:65, :], Ln)
                            nc.scalar.activation(st[64:65, QB:2 * QB], st[64:65, 0:QB],
                                                 Exp, scale=-1.0)
                            ps_b = pbc.tile([64, QB], F32)
                            nc.tensor.matmul(
                                ps_b[:],
                                lhsT=ones_sb[64:65, 0:64],
                                rhs=st[64:65, QB:2 * QB],
                                start=True, stop=True,
                            )
                            bc = stp.tile([64, QB], F32)
                            nc.scalar.copy(bc[:], ps_b[:])
                            if par == 0:
                                nc.vector.tensor_tensor(
                                    avt[hp][0:64, qb * QB: qb * QB + QB],
                                    ps_av[par][0:64, :], bc[:],
                                    mybir.AluOpType.mult)
                            else:
                                at = avp.tile([64, QB], DT)
                                nc.vector.tensor_tensor(
                                    at[:], ps_av[par][0:64, :], bc[:],
                                    mybir.AluOpType.mult)
                                nc.sync.dma_start(
                                    avt[hp][64:128, qb * QB: qb * QB + QB], at[:])

            # ---- phase 3: output projection (partial) ----
            with tc.tile_pool(name="ost", bufs=4) as ost, \
                 tc.tile_pool(name="psum_o", bufs=4, space="PSUM") as pso:
                for jc in range(8):
                    for nb in range(N_QB):
                        ps = pso.tile([128, QB], F32)
                        for ic in range(2):
                            nc.tensor.matmul(
                                ps[:],
                                lhsT=pwT_sb[:, ic * C + jc * 128: ic * C + (jc + 1) * 128],
                                rhs=avt[ic][:, nb * QB: nb * QB + QB],
                                start=(ic == 0), stop=(ic == 1),
                            )
                        o = ost.tile([128, QB], F32)
                        if (jc + nb) % 2:
                            nc.scalar.copy(o[:], ps[:])
                        else:
                            nc.vector.tensor_copy(o[:], ps[:])
                        nc.sync.dma_start(
                            out_e[jc * 128:(jc + 1) * 128, nb * QB: nb * QB + QB], o[:])

    _split_waits(nc)
    return nc


_NC_CACHE = {}


def _get_nc(dt_name=COMPUTE_DT):
    if dt_name not in _NC_CACHE:
        _NC_CACHE[dt_name] = build_kernel(dt_name=dt_name)
    return _NC_CACHE[dt_name]


def make_in_maps(x, padding_mask, alibi_bias, qkv_w, proj_w, dt_name=COMPUTE_DT):
    """Host-side sharding: returns list of 8 per-core input dicts."""
    np_dt = ml_dtypes.bfloat16 if dt_name == "bf16" else np.float32
    x = np.asarray(x, dtype=np.float32)
    padding_mask = np.asarray(padding_mask)
    alibi_bias = np.asarray(alibi_bias, dtype=np.float32)
    qkv_w = np.asarray(qkv_w, dtype=np.float32)
    proj_w = np.asarray(proj_w, dtype=np.float32)

    in_maps = []
    for core in range(NCORES):
        b, hg = divmod(core, 4)
        heads = [hg * H_CORE + j for j in range(H_CORE)]

        xT = np.ascontiguousarray(x[b].T).astype(np_dt)

        rows = []
        for qkv_i in range(3):
            for h in heads:
                rows.extend(range(qkv_i * C + h * HD, qkv_i * C + (h + 1) * HD))
        wqkvT = np.ascontiguousarray(qkv_w[rows].T)
        wqkvT[:, 0:H_CORE * HD] *= SCALE      # fold attention scale into Q
        wqkvT = wqkvT.astype(np_dt)

        mask_bias = np.where(padding_mask[b], np.float32(NEG_MASK),
                             np.float32(0.0)).astype(np.float32)
        ealibiT = np.empty((H_CORE * N, N), dtype=np_dt)
        for j, h in enumerate(heads):
            blk = alibi_bias[b, h].T + mask_bias[:, None]
            np.exp(blk, out=blk)
            ealibiT[j * N:(j + 1) * N] = blk.astype(np_dt)

        cols = []
        for h in heads:
            cols.extend(range(h * HD, (h + 1) * HD))
        pwT = np.ascontiguousarray(proj_w[:, cols].T).astype(np_dt)

        in_maps.append({"xT": xT, "wqkvT": wqkvT, "ealibiT": ealibiT, "pwT": pwT})
    return in_maps


def kernel(x, padding_mask, alibi_bias, qkv_w, proj_w, proj_b):
    nc = _get_nc()
    in_maps = make_in_maps(x, padding_mask, alibi_bias, qkv_w, proj_w)
    res = run_bass_kernel_spmd(nc, in_maps, core_ids=list(range(NCORES)))

    proj_b = np.asarray(proj_b, dtype=np.float32)
    out = np.empty((B, N, C), dtype=np.float32)
    for b in range(B):
        acc = res.results[b * 4 + 0]["out"].astype(np.float32)
        for g in range(1, 4):
            acc = acc + res.results[b * 4 + g]["out"]
        out[b] = acc.T + proj_b[None, :]
    return out
